# revision 2
# baseline (speedup 1.0000x reference)
"""RGCN (relational GCN) message-passing kernel for Trainium2, 8 NeuronCores.

Math (PyG RGCNConv, aggr='mean' per relation):
    out[i] = x[i] @ root + bias + sum_r W_r^T . mean_{j in N_r(i)} x[j]

Strategy:
  * Transform-first: z[n] = x[n] @ [W_0 | ... | W_7 | root]  -> per-node
    9 rows of 16 channels (8 relations + root). Then each edge only needs a
    16-wide gather of z[src, rel] and a weighted segment-sum over dst.
  * Shard nodes by dst across the 8 cores (contiguous 6250-node shards).
    Each core computes z for its own shard from a host-pretransposed x^T
    shard (pure matmul, no on-device transposes), then an AllGather
    replicates the z table (the small relation weights are replicated).
  * Host preprocessing (pure indexing, no FP math on the data path):
    per-core padded-CSR "rectangles" -- the dsts of a core are sorted by
    in-degree and packed into blocks of 128 (partition dim); each block is
    padded to its max degree K. A single weight array carries 1/count
    (mean normalization), 1.0 (root term) or 0.0 (padding).
  * Device: one indirect-DMA gather per chunk of blocks, a DVE multiply by
    the weights, and a DVE reduction over the K axis. Bias is added once at
    the end. Output rows return in rank-permuted order; the host inverse-
    permutes (pure indexing).
"""

import sys

sys.path.insert(0, "/opt/trn_rl_repo")

import numpy as np

N = 50000
E_EDGES = 600000
R = 8
DIN = 128
DOUT = 16
NCORES = 8
P = 128
NSH = N // NCORES            # 6250 nodes per shard
NBLK = (NSH + P - 1) // P    # 49 blocks of 128 dst slots
NSLOT = NBLK * P             # 6272 dst slots (incl. padding)
RPN = R + 1                  # rows per node in the z table (8 relations + root)
ZROWS_FLAT = NSLOT * RPN     # flat 16-wide rows per core table
MAX_DESC = 12000             # gather descriptors per chunk instruction

_CACHE = {}


def _host_prep(x, W, root, bias, edge_index, edge_type):
    src = np.asarray(edge_index[0]).astype(np.int64)
    dst = np.asarray(edge_index[1]).astype(np.int64)
    et = np.asarray(edge_type).astype(np.int64)

    # per-(dst, rel) counts -> per-edge mean weights  (index metadata)
    cnt = np.bincount(dst * R + et, minlength=N * R)
    inv = (1.0 / np.maximum(cnt, 1).astype(np.float32)).astype(np.float32)
    w_edge = inv[dst * R + et]

    # z-table flat row of each edge's (src, rel) entry, in AllGather layout
    zrow_edge = ((src // NSH) * NSLOT + (src % NSH)) * RPN + et

    owner = dst // NSH

    # pass 1: degree profile per core
    degs = []
    orders = []
    sels = []
    K_prof = np.zeros((NCORES, NBLK), np.int64)
    for k in range(NCORES):
        sel = np.nonzero(owner == k)[0]
        ld = dst[sel] - k * NSH
        deg = np.bincount(ld, minlength=NSH)
        order = np.argsort(-deg, kind="stable")  # ranked local dsts, deg desc
        deg_ranked = deg[order]
        # +1 for the root slot of every real dst
        K_prof[k] = deg_ranked[np.arange(NBLK) * P] + 1
        sels.append(sel)
        degs.append((deg, deg_ranked))
        orders.append(order)

    K_uni = K_prof.max(axis=0)
    off = np.zeros(NBLK + 1, np.int64)
    np.cumsum(K_uni, out=off[1:])
    S = int(off[-1])

    # chunks of consecutive equal-K blocks, descriptor-limited
    chunks = []  # (l0, nb, K, col_off)
    l = 0
    while l < NBLK:
        K = int(K_uni[l])
        l2 = l
        while l2 + 1 < NBLK and int(K_uni[l2 + 1]) == K \
                and (l2 + 2 - l) * K * P <= MAX_DESC:
            l2 += 1
        chunks.append((l, l2 - l + 1, K, int(off[l])))
        l = l2 + 1
    chunks = tuple(chunks)

    # pass 2: per-core rectangles
    in_maps = []
    Wfull = np.concatenate(
        [np.ascontiguousarray(W).transpose(1, 0, 2).reshape(DIN, R * DOUT),
         np.asarray(root, np.float32)], axis=1).astype(np.float32)
    bias_rep = np.broadcast_to(np.asarray(bias, np.float32), (P, DOUT)).copy()

    for k in range(NCORES):
        sel = sels[k]
        order = orders[k]
        deg, deg_ranked = degs[k]
        rank_of = np.empty(NSH, np.int64)
        rank_of[order] = np.arange(NSH)

        idx_arr = np.zeros((P, S), np.int32)
        wt_arr = np.zeros((P, S), np.float32)

        # edges
        ld = dst[sel] - k * NSH
        r_e = rank_of[ld]
        es = np.argsort(r_e, kind="stable")
        r_s = r_e[es]
        run_start = np.zeros(NSH + 1, np.int64)
        np.cumsum(deg_ranked, out=run_start[1:])
        j_s = np.arange(len(r_s)) - run_start[r_s]
        col = off[r_s // P] + j_s
        prow = r_s % P
        idx_arr[prow, col] = zrow_edge[sel][es].astype(np.int32)
        wt_arr[prow, col] = w_edge[sel][es]

        # root slots (one per real dst, right after its edges)
        s0 = np.arange(NSH)
        col_r = off[s0 // P] + deg_ranked[s0]
        prow_r = s0 % P
        idx_arr[prow_r, col_r] = ((k * NSLOT + order) * RPN + R).astype(np.int32)
        wt_arr[prow_r, col_r] = 1.0

        # x^T shard, padded
        xT = np.zeros((P, NSLOT), np.float32)
        xT[:, :NSH] = np.asarray(x[k * NSH:(k + 1) * NSH], np.float32).T

        in_maps.append({
            "xT": xT,
            "wfull": Wfull,
            "biasrep": bias_rep,
            "gidx": idx_arr,
            "gwt": wt_arr,
        })

    return in_maps, orders, S, chunks


def _build(S, chunks):
    import concourse.bacc as bacc
    import concourse.bass as bass
    import concourse.mybir as mybir
    import concourse.tile as tile

    f32 = mybir.dt.float32
    nc = bacc.Bacc("TRN2", target_bir_lowering=False, debug=False,
                   num_devices=NCORES)

    xT_in = nc.dram_tensor("xT", [P, NSLOT], f32, kind="ExternalInput")
    wf_in = nc.dram_tensor("wfull", [P, RPN * DOUT], f32, kind="ExternalInput")
    bias_in = nc.dram_tensor("biasrep", [P, DOUT], f32, kind="ExternalInput")
    idx_in = nc.dram_tensor("gidx", [P, S], mybir.dt.int32, kind="ExternalInput")
    wt_in = nc.dram_tensor("gwt", [P, S], f32, kind="ExternalInput")
    out_t = nc.dram_tensor("out", [P, NBLK * DOUT], f32, kind="ExternalOutput")

    with tile.TileContext(nc) as tc:
        with tc.tile_pool(name="const", bufs=1) as cpool, \
             tc.tile_pool(name="xt", bufs=4) as xpool, \
             tc.tile_pool(name="zps", bufs=4, space="PSUM") as pspool, \
             tc.tile_pool(name="zsb", bufs=4) as zpool, \
             tc.tile_pool(name="rect", bufs=1) as rpool, \
             tc.tile_pool(name="outp", bufs=1) as opool, \
             tc.tile_pool(name="dram", bufs=1, space="DRAM") as dram:

            wf_t = cpool.tile([P, RPN * DOUT], f32, tag="wf")
            nc.sync.dma_start(out=wf_t[:], in_=wf_in[:, :])
            bias_t = cpool.tile([P, DOUT], f32, tag="bias")
            nc.sync.dma_start(out=bias_t[:], in_=bias_in[:, :])
            idx_t = cpool.tile([P, S], mybir.dt.int32, tag="idx")
            nc.sync.dma_start(out=idx_t[:], in_=idx_in[:, :])
            wt_t = cpool.tile([P, S], f32, tag="wt")
            nc.sync.dma_start(out=wt_t[:], in_=wt_in[:, :])

            z_m = dram.tile([NSLOT, RPN * DOUT], f32)
            z_all = dram.tile([NCORES * NSLOT, RPN * DOUT], f32)

            # ---- transform: z = x^T.T @ [W|root] per 128-node tile ----
            for t in range(NBLK):
                xt = xpool.tile([P, P], f32, tag="xt")
                nc.sync.dma_start(out=xt[:], in_=xT_in[:, t * P:(t + 1) * P])
                ps = pspool.tile([P, RPN * DOUT], f32, tag="zps")
                nc.tensor.matmul(ps[:], lhsT=xt[:], rhs=wf_t[:],
                                 start=True, stop=True)
                zt = zpool.tile([P, RPN * DOUT], f32, tag="zsb")
                nc.scalar.copy(zt[:], ps[:])
                nc.sync.dma_start(out=z_m[t * P:(t + 1) * P, :], in_=zt[:])

            # ---- replicate the z table ----
            nc.gpsimd.collective_compute(
                "AllGather", mybir.AluOpType.bypass,
                replica_groups=[list(range(NCORES))],
                ins=[z_m.opt()], outs=[z_all.opt()],
            )
            z_flat = z_all[:, :].rearrange("n (r c) -> (n r) c", r=RPN, c=DOUT)

            partial = opool.tile([P, NBLK * DOUT], f32, tag="partial")

            # ---- gather + weighted segment reduce, chunk by chunk ----
            # HW indirect DMA contract: ONE index per partition per call,
            # each copying the partition's free size contiguously. So each
            # rect column (128 dst slots) is one gather call.
            for ci, (l0, nb, K, c0) in enumerate(chunks):
                ncols = nb * K
                rect = rpool.tile([P, ncols * DOUT], f32, tag=f"rect{ci}")
                for c in range(ncols):
                    nc.gpsimd.indirect_dma_start(
                        out=rect[:, c * DOUT:(c + 1) * DOUT],
                        out_offset=None,
                        in_=z_flat,
                        in_offset=bass.IndirectOffsetOnAxis(
                            ap=idx_t[:, c0 + c:c0 + c + 1], axis=0),
                    )
                rw = rpool.tile([P, ncols * DOUT], f32, tag=f"rw{ci}")
                # multiply by weights; write with K innermost for the reduce
                nc.vector.tensor_tensor(
                    out=rw[:].rearrange("p (nb c k) -> p nb k c", nb=nb, k=K, c=DOUT),
                    in0=rect[:].rearrange("p (nb k c) -> p nb k c", nb=nb, k=K, c=DOUT),
                    in1=wt_t[:, c0:c0 + ncols]
                        .rearrange("p (nb k) -> p nb k", nb=nb, k=K)
                        .unsqueeze(-1).to_broadcast((P, nb, K, DOUT)),
                    op=mybir.AluOpType.mult,
                )
                nc.vector.tensor_reduce(
                    out=partial[:, l0 * DOUT:(l0 + nb) * DOUT]
                        .rearrange("p (nb c) -> p nb c", c=DOUT),
                    in_=rw[:].rearrange("p (nb c k) -> p nb c k", nb=nb, k=K, c=DOUT),
                    axis=mybir.AxisListType.X,
                    op=mybir.AluOpType.add,
                )

            # ---- bias + store ----
            outt = opool.tile([P, NBLK * DOUT], f32, tag="outt")
            nc.vector.tensor_tensor(
                out=outt[:].rearrange("p (nb c) -> p nb c", c=DOUT),
                in0=partial[:].rearrange("p (nb c) -> p nb c", c=DOUT),
                in1=bias_t[:].unsqueeze(1).to_broadcast((P, NBLK, DOUT)),
                op=mybir.AluOpType.add,
            )
            nc.sync.dma_start(out=out_t[:, :], in_=outt[:])

    nc.compile()
    return nc


def kernel(x, W, root, bias, edge_index, edge_type, edge_ptr=None):
    from concourse import bass_utils

    in_maps, orders, S, chunks = _host_prep(x, W, root, bias,
                                            edge_index, edge_type)
    key = (S, chunks)
    if key not in _CACHE:
        _CACHE[key] = _build(S, chunks)
    nc = _CACHE[key]

    res = bass_utils.run_bass_kernel_spmd(nc, in_maps,
                                          core_ids=list(range(NCORES)))
    kernel.last_results = res

    out = np.empty((N, DOUT), np.float32)
    for k in range(NCORES):
        rows = (res.results[k]["out"]
                .reshape(P, NBLK, DOUT).transpose(1, 0, 2).reshape(NSLOT, DOUT))
        out[k * NSH + orders[k]] = rows[:NSH]
    return out


# revision 4
# speedup vs baseline: 1.0466x; 1.0466x over previous
"""RGCN (relational GCN) message-passing kernel for Trainium2, 8 NeuronCores.

Math (PyG RGCNConv, aggr='mean' per relation):
    out[i] = x[i] @ root + bias + sum_r W_r^T . mean_{j in N_r(i)} x[j]

Strategy:
  * Transform-first: z[n] = x[n] @ [W_0 | ... | W_7 | root]  -> per-node
    9 rows of 16 channels (8 relations + root). Then each edge only needs a
    16-wide gather of z[src, rel] and a weighted segment-sum over dst.
  * Shard nodes by dst across the 8 cores (contiguous 6250-node shards).
    Each core computes z for its own shard from a host-pretransposed x^T
    shard (pure matmul, no on-device transposes), then an AllGather
    replicates the z table (the small relation weights are replicated).
  * Host preprocessing (pure indexing, no FP math on the data path):
    per-core padded-CSR "rectangles" -- the dsts of a core are sorted by
    in-degree and packed into blocks of 128 (partition dim); each block is
    padded to its max degree K. A single weight array carries 1/count
    (mean normalization), 1.0 (root term) or 0.0 (padding).
  * Device: one indirect-DMA gather per chunk of blocks, a DVE multiply by
    the weights, and a DVE reduction over the K axis. Bias is added once at
    the end. Output rows return in rank-permuted order; the host inverse-
    permutes (pure indexing).
"""

import sys

sys.path.insert(0, "/opt/trn_rl_repo")

import numpy as np

N = 50000
E_EDGES = 600000
R = 8
DIN = 128
DOUT = 16
NCORES = 8
P = 128
NSH = N // NCORES            # 6250 nodes per shard
NBLK = (NSH + P - 1) // P    # 49 blocks of 128 dst slots
NSLOT = NBLK * P             # 6272 dst slots (incl. padding)
RPN = R + 1                  # rows per node in the z table (8 relations + root)
ZROWS_FLAT = NSLOT * RPN     # flat 16-wide rows per core table
MAX_DESC = 12000             # gather descriptors per chunk instruction

_CACHE = {}


def _host_prep(x, W, root, bias, edge_index, edge_type):
    src = np.asarray(edge_index[0]).astype(np.int64)
    dst = np.asarray(edge_index[1]).astype(np.int64)
    et = np.asarray(edge_type).astype(np.int64)

    # per-(dst, rel) counts -> per-edge mean weights  (index metadata)
    cnt = np.bincount(dst * R + et, minlength=N * R)
    inv = (1.0 / np.maximum(cnt, 1).astype(np.float32)).astype(np.float32)
    w_edge = inv[dst * R + et]

    # z-table flat row of each edge's (src, rel) entry, in AllGather layout
    zrow_edge = ((src // NSH) * NSLOT + (src % NSH)) * RPN + et

    owner = dst // NSH

    # pass 1: degree profile per core
    degs = []
    orders = []
    sels = []
    K_prof = np.zeros((NCORES, NBLK), np.int64)
    for k in range(NCORES):
        sel = np.nonzero(owner == k)[0]
        ld = dst[sel] - k * NSH
        deg = np.bincount(ld, minlength=NSH)
        order = np.argsort(-deg, kind="stable")  # ranked local dsts, deg desc
        deg_ranked = deg[order]
        # +1 for the root slot of every real dst
        K_prof[k] = deg_ranked[np.arange(NBLK) * P] + 1
        sels.append(sel)
        degs.append((deg, deg_ranked))
        orders.append(order)

    K_uni = K_prof.max(axis=0)
    off = np.zeros(NBLK + 1, np.int64)
    np.cumsum(K_uni, out=off[1:])
    S = int(off[-1])

    # chunks of consecutive equal-K blocks, descriptor-limited
    chunks = []  # (l0, nb, K, col_off)
    l = 0
    while l < NBLK:
        K = int(K_uni[l])
        l2 = l
        while l2 + 1 < NBLK and int(K_uni[l2 + 1]) == K \
                and (l2 + 2 - l) * K * P <= MAX_DESC:
            l2 += 1
        chunks.append((l, l2 - l + 1, K, int(off[l])))
        l = l2 + 1
    chunks = tuple(chunks)

    # pass 2: per-core rectangles
    in_maps = []
    Wfull = np.concatenate(
        [np.ascontiguousarray(W).transpose(1, 0, 2).reshape(DIN, R * DOUT),
         np.asarray(root, np.float32)], axis=1).astype(np.float32)
    bias_rep = np.broadcast_to(np.asarray(bias, np.float32), (P, DOUT)).copy()

    for k in range(NCORES):
        sel = sels[k]
        order = orders[k]
        deg, deg_ranked = degs[k]
        rank_of = np.empty(NSH, np.int64)
        rank_of[order] = np.arange(NSH)

        idx_arr = np.zeros((P, S), np.int32)
        wt_arr = np.zeros((P, S), np.float32)

        # edges
        ld = dst[sel] - k * NSH
        r_e = rank_of[ld]
        es = np.argsort(r_e, kind="stable")
        r_s = r_e[es]
        run_start = np.zeros(NSH + 1, np.int64)
        np.cumsum(deg_ranked, out=run_start[1:])
        j_s = np.arange(len(r_s)) - run_start[r_s]
        col = off[r_s // P] + j_s
        prow = r_s % P
        idx_arr[prow, col] = zrow_edge[sel][es].astype(np.int32)
        wt_arr[prow, col] = w_edge[sel][es]

        # root slots (one per real dst, right after its edges)
        s0 = np.arange(NSH)
        col_r = off[s0 // P] + deg_ranked[s0]
        prow_r = s0 % P
        idx_arr[prow_r, col_r] = ((k * NSLOT + order) * RPN + R).astype(np.int32)
        wt_arr[prow_r, col_r] = 1.0

        # x^T shard, padded
        xT = np.zeros((P, NSLOT), np.float32)
        xT[:, :NSH] = np.asarray(x[k * NSH:(k + 1) * NSH], np.float32).T

        in_maps.append({
            "xT": xT,
            "wfull": Wfull,
            "biasrep": bias_rep,
            "gidx": idx_arr,
            "gwt": wt_arr,
        })

    return in_maps, orders, S, chunks


def _build(S, chunks):
    import concourse.bacc as bacc
    import concourse.bass as bass
    import concourse.mybir as mybir
    import concourse.tile as tile

    f32 = mybir.dt.float32
    nc = bacc.Bacc("TRN2", target_bir_lowering=False, debug=False,
                   num_devices=NCORES)

    xT_in = nc.dram_tensor("xT", [P, NSLOT], f32, kind="ExternalInput")
    wf_in = nc.dram_tensor("wfull", [P, RPN * DOUT], f32, kind="ExternalInput")
    bias_in = nc.dram_tensor("biasrep", [P, DOUT], f32, kind="ExternalInput")
    idx_in = nc.dram_tensor("gidx", [P, S], mybir.dt.int32, kind="ExternalInput")
    wt_in = nc.dram_tensor("gwt", [P, S], f32, kind="ExternalInput")
    out_t = nc.dram_tensor("out", [P, NBLK * DOUT], f32, kind="ExternalOutput")

    with tile.TileContext(nc) as tc:
        with tc.tile_pool(name="const", bufs=1) as cpool, \
             tc.tile_pool(name="xt", bufs=1) as xpool, \
             tc.tile_pool(name="zps", bufs=4, space="PSUM") as pspool, \
             tc.tile_pool(name="zsb", bufs=4) as zpool, \
             tc.tile_pool(name="rect", bufs=1) as rpool, \
             tc.tile_pool(name="outp", bufs=1) as opool, \
             tc.tile_pool(name="dram", bufs=1, space="DRAM") as dram:

            wf_t = cpool.tile([P, RPN * DOUT], f32, tag="wf")
            nc.sync.dma_start(out=wf_t[:], in_=wf_in[:, :])
            bias_t = cpool.tile([P, DOUT], f32, tag="bias")
            nc.sync.dma_start(out=bias_t[:], in_=bias_in[:, :])
            idx_t = cpool.tile([P, S], mybir.dt.int32, tag="idx")
            nc.sync.dma_start(out=idx_t[:], in_=idx_in[:, :])
            wt_t = cpool.tile([P, S], f32, tag="wt")
            nc.sync.dma_start(out=wt_t[:], in_=wt_in[:, :])

            z_m = dram.tile([NSLOT, RPN * DOUT], f32)
            z_all = dram.tile([NCORES * NSLOT, RPN * DOUT], f32)

            # ---- transform: z = x^T.T @ [W|root] per 128-node tile ----
            xt_big = xpool.tile([P, NSLOT], f32, tag="xtb")
            nc.sync.dma_start(out=xt_big[:], in_=xT_in[:, :])
            for t in range(NBLK):
                ps = pspool.tile([P, RPN * DOUT], f32, tag="zps")
                nc.tensor.matmul(ps[:], lhsT=xt_big[:, t * P:(t + 1) * P],
                                 rhs=wf_t[:], start=True, stop=True)
                zt = zpool.tile([P, RPN * DOUT], f32, tag="zsb")
                nc.scalar.copy(zt[:], ps[:])
                nc.sync.dma_start(out=z_m[t * P:(t + 1) * P, :], in_=zt[:])

            # ---- replicate the z table ----
            nc.gpsimd.collective_compute(
                "AllGather", mybir.AluOpType.bypass,
                replica_groups=[list(range(NCORES))],
                ins=[z_m.opt()], outs=[z_all.opt()],
            )
            z_flat = z_all[:, :].rearrange("n (r c) -> (n r) c", r=RPN, c=DOUT)

            partial = opool.tile([P, NBLK * DOUT], f32, tag="partial")

            # ---- gather + weighted segment reduce, chunk by chunk ----
            # HW indirect DMA contract: ONE index per partition per call,
            # each copying the partition's free size contiguously. So each
            # rect column (128 dst slots) is one gather call.
            for ci, (l0, nb, K, c0) in enumerate(chunks):
                ncols = nb * K
                rect = rpool.tile([P, ncols * DOUT], f32, tag=f"rect{ci}")
                for c in range(ncols):
                    nc.gpsimd.indirect_dma_start(
                        out=rect[:, c * DOUT:(c + 1) * DOUT],
                        out_offset=None,
                        in_=z_flat,
                        in_offset=bass.IndirectOffsetOnAxis(
                            ap=idx_t[:, c0 + c:c0 + c + 1], axis=0),
                    )
                rw = rpool.tile([P, ncols * DOUT], f32, tag=f"rw{ci}")
                # multiply by weights; write with K innermost for the reduce
                nc.vector.tensor_tensor(
                    out=rw[:].rearrange("p (nb c k) -> p nb k c", nb=nb, k=K, c=DOUT),
                    in0=rect[:].rearrange("p (nb k c) -> p nb k c", nb=nb, k=K, c=DOUT),
                    in1=wt_t[:, c0:c0 + ncols]
                        .rearrange("p (nb k) -> p nb k", nb=nb, k=K)
                        .unsqueeze(-1).to_broadcast((P, nb, K, DOUT)),
                    op=mybir.AluOpType.mult,
                )
                nc.vector.tensor_reduce(
                    out=partial[:, l0 * DOUT:(l0 + nb) * DOUT]
                        .rearrange("p (nb c) -> p nb c", c=DOUT),
                    in_=rw[:].rearrange("p (nb c k) -> p nb c k", nb=nb, k=K, c=DOUT),
                    axis=mybir.AxisListType.X,
                    op=mybir.AluOpType.add,
                )

            # ---- bias + store ----
            outt = opool.tile([P, NBLK * DOUT], f32, tag="outt")
            nc.vector.tensor_tensor(
                out=outt[:].rearrange("p (nb c) -> p nb c", c=DOUT),
                in0=partial[:].rearrange("p (nb c) -> p nb c", c=DOUT),
                in1=bias_t[:].unsqueeze(1).to_broadcast((P, NBLK, DOUT)),
                op=mybir.AluOpType.add,
            )
            nc.sync.dma_start(out=out_t[:, :], in_=outt[:])

    nc.compile()
    return nc


def kernel(x, W, root, bias, edge_index, edge_type, edge_ptr=None):
    from concourse import bass_utils

    in_maps, orders, S, chunks = _host_prep(x, W, root, bias,
                                            edge_index, edge_type)
    key = (S, chunks)
    if key not in _CACHE:
        _CACHE[key] = _build(S, chunks)
    nc = _CACHE[key]

    res = bass_utils.run_bass_kernel_spmd(nc, in_maps,
                                          core_ids=list(range(NCORES)))
    kernel.last_results = res

    out = np.empty((N, DOUT), np.float32)
    for k in range(NCORES):
        rows = (res.results[k]["out"]
                .reshape(P, NBLK, DOUT).transpose(1, 0, 2).reshape(NSLOT, DOUT))
        out[k * NSH + orders[k]] = rows[:NSH]
    return out
